# revision 28
# baseline (speedup 1.0000x reference)
"""Trainium2 Bass kernel for block-local (sparse) attention with relative
position embeddings.

Problem (hardcoded): bs=8, n=8192, dim=512, heads=8, dim_head=64,
context_size=256 -> 32 independent 256-token blocks per batch element.

Sharding: pure data-parallel over batch -- core i computes batch element i.
Weights are replicated; no collectives.

Device pipeline (per core), v10:
  - x arrives host-pre-transposed/bf16 as xT [512, 8192]; Wq pre-scaled.
  - qT/kT feature-major (lhsT=W), v token-major (lhsT=xT), all bf16 via PE.
  - Relative position: P2 windows = q @ E2T-window per (head, c-tile); staged
    to DRAM and read back with a plain sheared 4D-AP DMA as pos[c, r].
  - dotsT[r, c] = kT.T@qT accumulates pos via transpose-as-matmul
    (identity rhs, start=False) directly in PSUM; ACT exps PSUM->SBUF giving
    attnT bf16 ready as av lhsT.
  - av rhs = [v_h | ones]: softmax denominator S[c] rides column 64;
    DVE reciprocal + broadcast multiply normalizes token-major AO;
    PE transposes AO back feature-major; out-proj adds bout via ACT bias.
  - yT [512, 8192] f32 stored; host transposes back.
  Scheduling: engines run their streams in static order, so the emitter
  interleaves three phases (projections of superblock s, pos-pipeline of s,
  attention of s-1) at work-item granularity to avoid head-of-line stalls.
"""

from contextlib import ExitStack

import numpy as np

HEADS = 8
DH = 64
DIM = 512
C = 256
MAX_POS_EMB = 512
BS = 8
N_TOK = 8192
NB_FULL = N_TOK // C  # 32 blocks

_BF16 = None


def _bf16():
    global _BF16
    if _BF16 is None:
        import ml_dtypes

        _BF16 = np.dtype(ml_dtypes.bfloat16)
    return _BF16


def build_nc(nb):
    """Build the Bass graph for nb blocks (nb*256 tokens) per core."""
    import concourse.bass as bass
    import concourse.tile as tile
    from concourse import bacc, mybir
    from concourse.ap import AP

    assert nb % 2 == 0
    nsb = nb // 2  # superblocks of 512 tokens
    ntok = nb * C

    bf16 = mybir.dt.bfloat16
    f32 = mybir.dt.float32
    EXP = mybir.ActivationFunctionType.Exp
    IDENT = mybir.ActivationFunctionType.Identity

    nc = bacc.Bacc("TRN2", target_bir_lowering=False, debug=False, num_devices=8)

    xt_d = nc.dram_tensor("xt", [DIM, ntok], bf16, kind="ExternalInput")
    wq_d = nc.dram_tensor("wq", [DIM, DIM], bf16, kind="ExternalInput")
    wk_d = nc.dram_tensor("wk", [DIM, DIM], bf16, kind="ExternalInput")
    wv_d = nc.dram_tensor("wv", [DIM, DIM], bf16, kind="ExternalInput")
    wout_d = nc.dram_tensor("wout", [DIM, DIM], bf16, kind="ExternalInput")
    e2t_d = nc.dram_tensor("e2t", [128, 512], bf16, kind="ExternalInput")
    ident_d = nc.dram_tensor("ident", [128, 128], bf16, kind="ExternalInput")
    bout_d = nc.dram_tensor("boutt", [128, 4], f32, kind="ExternalInput")
    yt_d = nc.dram_tensor("yt", [DIM, ntok], f32, kind="ExternalOutput")
    # scratch for the relative-position shear; layout [blk][p][h][ct][jl]
    # matches the SBUF staging tile so stores are fully contiguous
    p2s_d = nc.dram_tensor("p2s", [nb, 128, HEADS, 2, 384], bf16)
    S_P = HEADS * 2 * 384
    S_BLK = 128 * S_P

    with tile.TileContext(nc) as tc, ExitStack() as ctx:
        const = ctx.enter_context(tc.tile_pool(name="const", bufs=1))
        xpool = ctx.enter_context(tc.tile_pool(name="xp", bufs=2))
        qpool = ctx.enter_context(tc.tile_pool(name="qp", bufs=2))
        kpool = ctx.enter_context(tc.tile_pool(name="kp", bufs=2))
        vpool = ctx.enter_context(tc.tile_pool(name="vp", bufs=2))
        p2stpool = ctx.enter_context(tc.tile_pool(name="p2st", bufs=2))
        pospool = ctx.enter_context(tc.tile_pool(name="pos", bufs=4))
        atpool = ctx.enter_context(tc.tile_pool(name="at", bufs=6))
        recpool = ctx.enter_context(tc.tile_pool(name="rec", bufs=2))
        aopool = ctx.enter_context(tc.tile_pool(name="ao", bufs=2))
        aotpool = ctx.enter_context(tc.tile_pool(name="aot", bufs=2))
        ypool = ctx.enter_context(tc.tile_pool(name="yp", bufs=2))
        psA = ctx.enter_context(
            tc.tile_pool(name="psA", bufs=2, space=bass.MemorySpace.PSUM)
        )
        psD = ctx.enter_context(
            tc.tile_pool(name="psD", bufs=3, space=bass.MemorySpace.PSUM)
        )
        psP = ctx.enter_context(
            tc.tile_pool(name="psP", bufs=2, space=bass.MemorySpace.PSUM)
        )
        psAO = ctx.enter_context(
            tc.tile_pool(name="psAO", bufs=1, space=bass.MemorySpace.PSUM)
        )

        # ---- resident constants ----
        wq_sb = const.tile([128, 4, DIM], bf16)
        wk_sb = const.tile([128, 4, DIM], bf16)
        wv_sb = const.tile([128, 4, DIM], bf16)
        wout_sb = const.tile([128, 4, DIM], bf16)
        for k4 in range(4):
            nc.sync.dma_start(wq_sb[:, k4, :], wq_d[k4 * 128 : (k4 + 1) * 128, :])
            nc.sync.dma_start(wk_sb[:, k4, :], wk_d[k4 * 128 : (k4 + 1) * 128, :])
            nc.sync.dma_start(wv_sb[:, k4, :], wv_d[k4 * 128 : (k4 + 1) * 128, :])
            nc.sync.dma_start(wout_sb[:, k4, :], wout_d[k4 * 128 : (k4 + 1) * 128, :])
        e2t_sb = const.tile([128, 512], bf16)
        nc.sync.dma_start(e2t_sb[:], e2t_d[:])
        ident_sb = const.tile([128, 128], bf16)
        nc.sync.dma_start(ident_sb[:], ident_d[:])
        bout_sb = const.tile([128, 4], f32)
        nc.sync.dma_start(bout_sb[:], bout_d[:])

        def proj_gen(s, xt_t, qt, kt, v_sb):
            """Projections of superblock s; yields per PSUM group."""
            for m in range(4):
                psq = psA.tile([128, 512], f32, tag="psa", name="psq")
                for k4 in range(4):
                    nc.tensor.matmul(
                        psq[:],
                        wq_sb[:, k4, m * 128 : (m + 1) * 128],
                        xt_t[:, k4, :],
                        start=(k4 == 0),
                        stop=(k4 == 3),
                    )
                nc.scalar.copy(qt[:, m, :], psq[:])
                yield
            for m in range(4):
                psk = psA.tile([128, 512], f32, tag="psa", name="psk")
                for k4 in range(4):
                    nc.tensor.matmul(
                        psk[:],
                        wk_sb[:, k4, m * 128 : (m + 1) * 128],
                        xt_t[:, k4, :],
                        start=(k4 == 0),
                        stop=(k4 == 3),
                    )
                nc.vector.tensor_copy(kt[:, m, :], psk[:])
                yield
            nc.vector.memset(v_sb[:, :, :, 64:65], 1.0)
            for mt in range(4):
                psv = psA.tile([128, 512], f32, tag="psa", name="psv")
                for k4 in range(4):
                    nc.tensor.matmul(
                        psv[:],
                        xt_t[:, k4, mt * 128 : (mt + 1) * 128],
                        wv_sb[:, k4, :],
                        start=(k4 == 0),
                        stop=(k4 == 3),
                    )
                nc.vector.tensor_copy(
                    v_sb[:, mt, :, 0:64],
                    psv[:].rearrange("p (h e) -> p h e", h=HEADS),
                )
                yield

        def p1_gen(s, qt, post_blocks):
            """Relative-position pipeline of superblock s (both blocks).
            Yields after each matmul+copy pair."""
            for b in range(2):
                blk = 2 * s + b
                p2stage = p2stpool.tile([128, HEADS, 2, 384], bf16)
                post_all = post_blocks[b]
                for ct in range(2):
                    for hp in range(HEADS // 2):
                        # emit head pairs back-to-back so their alternating
                        # row groups let LDWEIGHTS pull ahead on the PE
                        pair_ps = []
                        for h in (2 * hp, 2 * hp + 1):
                            bp = (h % 2) * 64
                            m4 = h // 2
                            p2ps = psP.tile(
                                [128, 384], f32, tag="psp", name=f"p2ps{h % 2}"
                            )
                            nc.tensor.matmul(
                                p2ps[:],
                                qt[
                                    bp : bp + 64,
                                    m4,
                                    b * 256 + ct * 128 : b * 256 + ct * 128 + 128,
                                ],
                                e2t_sb[
                                    bp : bp + 64,
                                    (1 - ct) * 128 : (1 - ct) * 128 + 384,
                                ],
                                tile_position=(bp, 0),
                            )
                            pair_ps.append((h, p2ps))
                        for h, p2ps in pair_ps:
                            if (h + ct) % 2 == 0:
                                nc.scalar.copy(p2stage[:, h, ct, :], p2ps[:])
                            else:
                                nc.vector.tensor_copy(p2stage[:, h, ct, :], p2ps[:])
                        yield
                    # store + sheared read per ct half (scratch layout matches
                    # staging -> contiguous store runs)
                    nc.gpsimd.dma_start(
                        AP(
                            p2s_d,
                            blk * S_BLK + ct * 384,
                            [[S_P, 128], [2 * 384, HEADS], [1, 384]],
                        ),
                        p2stage[:, :, ct, :],
                    )
                    nc.sync.dma_start(
                        post_all[:, :, ct, :],
                        AP(
                            p2s_d,
                            blk * S_BLK + ct * 384 + 127,
                            [[S_P - 1, 128], [2 * 384, HEADS], [1, 256]],
                        ),
                    )
                    yield

        def attn_gen(st):
            """Attention phase for a staged superblock; yields per work item."""
            s, qt, kt, v_sb, post_blocks = st
            aot_sb = aotpool.tile([128, 4, 512], bf16)
            for b in range(2):
                post_all = post_blocks[b]
                ao_sb = aopool.tile([128, 2, HEADS, 64], bf16)
                for g in range(2):  # head groups of 4
                    att_group = []
                    for hp in range(2):  # head pairs (row groups alternate)
                        hpair = (4 * g + 2 * hp, 4 * g + 2 * hp + 1)
                        dts = {}
                        for h in hpair:
                            dts[h] = psD.tile(
                                [128, 2, 256], f32, tag="psd", name=f"dt{h % 2}"
                            )
                        for rt in range(2):
                            for h in hpair:
                                bp = (h % 2) * 64
                                m4 = h // 2
                                nc.tensor.matmul(
                                    dts[h][:, rt, :],
                                    kt[
                                        bp : bp + 64,
                                        m4,
                                        b * 256 + rt * 128 : b * 256 + rt * 128 + 128,
                                    ],
                                    qt[bp : bp + 64, m4, b * 256 : (b + 1) * 256],
                                    start=(rt == 0),
                                    stop=False,
                                    tile_position=(bp, 0),
                                    skip_group_check=True,
                                )
                        yield
                        for h in hpair:
                            dt_ps = dts[h]
                            for ct in range(2):
                                for rt in range(2):
                                    nc.tensor.matmul(
                                        dt_ps[:, rt, ct * 128 : ct * 128 + 128],
                                        post_all[:, h, ct, rt * 128 : rt * 128 + 128],
                                        ident_sb[:],
                                        start=False,
                                        stop=(ct == 1 and rt == 1),
                                        skip_group_check=True,
                                    )
                            att_sb = atpool.tile([128, 2, 256], bf16)
                            nc.scalar.activation(att_sb[:], dt_ps[:], EXP)
                            att_group.append(att_sb)
                            yield
                    for ct in range(2):
                        ao = psAO.tile([128, 4, 65], f32, tag="psao")
                        for hh in range(4):
                            h = 4 * g + hh
                            for rt in range(2):
                                nc.tensor.matmul(
                                    ao[:, hh, :],
                                    att_group[hh][:, rt, ct * 128 : ct * 128 + 128],
                                    v_sb[:, b * 2 + rt, h, :],
                                    start=(rt == 0),
                                    stop=(rt == 1),
                                )
                        rec = recpool.tile([128, 4], f32)
                        nc.vector.reciprocal(rec[:], ao[:, :, 64])
                        rec_b = rec[:].unsqueeze(2).to_broadcast([128, 4, 64])
                        nc.vector.tensor_mul(
                            ao_sb[:, ct, 4 * g : 4 * g + 4, :],
                            ao[:, :, 0:64],
                            rec_b,
                        )
                        yield
                # transpose AO back to feature-major
                for ct in range(2):
                    for it in range(4):
                        tp = psD.tile([128, 128], f32, tag="psd", name="tp")
                        nc.tensor.matmul(
                            tp[:],
                            ao_sb[:, ct, 2 * it : 2 * it + 2, :].rearrange(
                                "p a b -> p (a b)"
                            ),
                            ident_sb[:],
                        )
                        nc.vector.tensor_copy(
                            aot_sb[
                                :, it, b * 256 + ct * 128 : b * 256 + ct * 128 + 128
                            ],
                            tp[:],
                        )
                        if it % 2 == 1:
                            yield
            # output projection
            yt_t = ypool.tile([128, 4, 512], f32)
            for m in range(4):
                psy = psA.tile([128, 512], f32, tag="psa", name="psy")
                for k4 in range(4):
                    nc.tensor.matmul(
                        psy[:],
                        wout_sb[:, k4, m * 128 : (m + 1) * 128],
                        aot_sb[:, k4, :],
                        start=(k4 == 0),
                        stop=(k4 == 3),
                    )
                nc.scalar.activation(
                    yt_t[:, m, :], psy[:], IDENT, bias=bout_sb[:, m : m + 1]
                )
                yield
            nc.gpsimd.dma_start(
                AP(yt_d, s * 512, [[ntok, 128], [128 * ntok, 4], [1, 512]]),
                yt_t[:],
            )

        def drive(gens):
            """Round-robin the generators until all are exhausted."""
            gens = [g for g in gens if g is not None]
            while gens:
                nxt = []
                for g in gens:
                    try:
                        next(g)
                        nxt.append(g)
                    except StopIteration:
                        pass
                gens = nxt

        staged = None
        for s in range(nsb):
            xt_t = xpool.tile([128, 4, 512], bf16)
            nc.gpsimd.dma_start(
                xt_t[:],
                AP(xt_d, s * 512, [[ntok, 128], [128 * ntok, 4], [1, 512]]),
            )
            qt = qpool.tile([128, 4, 512], bf16)
            kt = kpool.tile([128, 4, 512], bf16)
            v_sb = vpool.tile([128, 4, HEADS, 65], bf16)
            post_blocks = [
                pospool.tile([128, HEADS, 2, 256], bf16, name=f"post{b}")
                for b in range(2)
            ]
            g_proj = proj_gen(s, xt_t, qt, kt, v_sb)
            g_p1 = p1_gen(s, qt, post_blocks)
            g_attn = attn_gen(staged) if staged is not None else None
            drive([g_proj, g_p1, g_attn])
            staged = (s, qt, kt, v_sb, post_blocks)
        drive([attn_gen(staged)])

    nc.compile()
    return nc


def prep_host_inputs(x, Wq, Wkv, Wout, bout, rel_emb, nb):
    """Build per-core input maps (host-side layout prep)."""
    bf = _bf16()
    scale = DH ** -0.5
    ntok = nb * C
    wq = np.ascontiguousarray((Wq * scale)).astype(bf)
    wk = np.ascontiguousarray(Wkv[:, :DIM]).astype(bf)
    wv = np.ascontiguousarray(Wkv[:, DIM:]).astype(bf)
    wout = np.ascontiguousarray(Wout).astype(bf)
    # e2t[d, j] = rel_emb[767 - j, d], j in [0, 511); duplicated on rows 64-127
    e2t = np.zeros((128, 512), dtype=bf)
    block = rel_emb[767:256:-1, :].T.astype(bf)  # [64, 511]
    e2t[0:64, 0:511] = block
    e2t[64:128, 0:511] = block
    ident = np.eye(128, dtype=np.float32).astype(bf)
    boutt = np.ascontiguousarray(bout.reshape(4, 128).T).astype(np.float32)
    in_maps = []
    for i in range(BS):
        xt = np.ascontiguousarray(x[i, :ntok, :].T).astype(bf)
        in_maps.append(
            {
                "xt": xt,
                "wq": wq,
                "wk": wk,
                "wv": wv,
                "wout": wout,
                "e2t": e2t,
                "ident": ident,
                "boutt": boutt,
            }
        )
    return in_maps


_NC_CACHE = {}


def _get_nc(nb):
    if nb not in _NC_CACHE:
        _NC_CACHE[nb] = build_nc(nb)
    return _NC_CACHE[nb]


def kernel(x, Wq, Wkv, Wout, bout, rel_emb, context_size):
    from concourse.bass_utils import run_bass_kernel_spmd

    x = np.asarray(x, dtype=np.float32)
    Wq = np.asarray(Wq, dtype=np.float32)
    Wkv = np.asarray(Wkv, dtype=np.float32)
    Wout = np.asarray(Wout, dtype=np.float32)
    bout = np.asarray(bout, dtype=np.float32)
    rel_emb = np.asarray(rel_emb, dtype=np.float32)
    assert int(context_size) == C
    assert x.shape == (BS, N_TOK, DIM)

    nb = NB_FULL
    nc = _get_nc(nb)
    in_maps = prep_host_inputs(x, Wq, Wkv, Wout, bout, rel_emb, nb)
    res = run_bass_kernel_spmd(nc, in_maps, core_ids=list(range(BS)))
    out = np.empty((BS, N_TOK, DIM), dtype=np.float32)
    for i in range(BS):
        out[i] = res.results[i]["yt"].T
    return out


# revision 30
# speedup vs baseline: 1.0362x; 1.0362x over previous
"""Trainium2 Bass kernel for block-local (sparse) attention with relative
position embeddings.

Problem (hardcoded): bs=8, n=8192, dim=512, heads=8, dim_head=64,
context_size=256 -> 32 independent 256-token blocks per batch element.

Sharding: pure data-parallel over batch -- core i computes batch element i.
Weights are replicated; no collectives.

Device pipeline (per core), v10:
  - x arrives host-pre-transposed/bf16 as xT [512, 8192]; Wq pre-scaled.
  - qT/kT feature-major (lhsT=W), v token-major (lhsT=xT), all bf16 via PE.
  - Relative position: P2 windows = q @ E2T-window per (head, c-tile); staged
    to DRAM and read back with a plain sheared 4D-AP DMA as pos[c, r].
  - dotsT[r, c] = kT.T@qT accumulates pos via transpose-as-matmul
    (identity rhs, start=False) directly in PSUM; ACT exps PSUM->SBUF giving
    attnT bf16 ready as av lhsT.
  - av rhs = [v_h | ones]: softmax denominator S[c] rides column 64;
    DVE reciprocal + broadcast multiply normalizes token-major AO;
    PE transposes AO back feature-major; out-proj adds bout via ACT bias.
  - yT [512, 8192] f32 stored; host transposes back.
  Scheduling: engines run their streams in static order, so the emitter
  interleaves three phases (projections of superblock s, pos-pipeline of s,
  attention of s-1) at work-item granularity to avoid head-of-line stalls.
"""

from contextlib import ExitStack

import numpy as np

HEADS = 8
DH = 64
DIM = 512
C = 256
MAX_POS_EMB = 512
BS = 8
N_TOK = 8192
NB_FULL = N_TOK // C  # 32 blocks

_BF16 = None


def _bf16():
    global _BF16
    if _BF16 is None:
        import ml_dtypes

        _BF16 = np.dtype(ml_dtypes.bfloat16)
    return _BF16


def build_nc(nb):
    """Build the Bass graph for nb blocks (nb*256 tokens) per core."""
    import concourse.bass as bass
    import concourse.tile as tile
    from concourse import bacc, mybir
    from concourse.ap import AP

    assert nb % 2 == 0
    nsb = nb // 2  # superblocks of 512 tokens
    ntok = nb * C

    bf16 = mybir.dt.bfloat16
    f32 = mybir.dt.float32
    EXP = mybir.ActivationFunctionType.Exp
    IDENT = mybir.ActivationFunctionType.Identity

    nc = bacc.Bacc("TRN2", target_bir_lowering=False, debug=False, num_devices=8)

    xt_d = nc.dram_tensor("xt", [DIM, ntok], bf16, kind="ExternalInput")
    wq_d = nc.dram_tensor("wq", [DIM, DIM], bf16, kind="ExternalInput")
    wk_d = nc.dram_tensor("wk", [DIM, DIM], bf16, kind="ExternalInput")
    wv_d = nc.dram_tensor("wv", [DIM, DIM], bf16, kind="ExternalInput")
    wout_d = nc.dram_tensor("wout", [DIM, DIM], bf16, kind="ExternalInput")
    e2t_d = nc.dram_tensor("e2t", [128, 512], bf16, kind="ExternalInput")
    ident_d = nc.dram_tensor("ident", [128, 128], bf16, kind="ExternalInput")
    bout_d = nc.dram_tensor("boutt", [128, 4], f32, kind="ExternalInput")
    yt_d = nc.dram_tensor("yt", [DIM, ntok], f32, kind="ExternalOutput")
    # scratch for the relative-position shear; layout [blk][p][h][ct][jl]
    # matches the SBUF staging tile so stores are fully contiguous
    p2s_d = nc.dram_tensor("p2s", [nb, 128, HEADS, 2, 384], bf16)
    S_P = HEADS * 2 * 384
    S_BLK = 128 * S_P

    with tile.TileContext(nc) as tc, ExitStack() as ctx:
        const = ctx.enter_context(tc.tile_pool(name="const", bufs=1))
        xpool = ctx.enter_context(tc.tile_pool(name="xp", bufs=2))
        qpool = ctx.enter_context(tc.tile_pool(name="qp", bufs=2))
        kpool = ctx.enter_context(tc.tile_pool(name="kp", bufs=2))
        vpool = ctx.enter_context(tc.tile_pool(name="vp", bufs=2))
        p2stpool = ctx.enter_context(tc.tile_pool(name="p2st", bufs=2))
        pospool = ctx.enter_context(tc.tile_pool(name="pos", bufs=5))
        atpool = ctx.enter_context(tc.tile_pool(name="at", bufs=8))
        recpool = ctx.enter_context(tc.tile_pool(name="rec", bufs=2))
        aopool = ctx.enter_context(tc.tile_pool(name="ao", bufs=2))
        aotpool = ctx.enter_context(tc.tile_pool(name="aot", bufs=2))
        ypool = ctx.enter_context(tc.tile_pool(name="yp", bufs=2))
        psA = ctx.enter_context(
            tc.tile_pool(name="psA", bufs=2, space=bass.MemorySpace.PSUM)
        )
        psD = ctx.enter_context(
            tc.tile_pool(name="psD", bufs=3, space=bass.MemorySpace.PSUM)
        )
        psP = ctx.enter_context(
            tc.tile_pool(name="psP", bufs=2, space=bass.MemorySpace.PSUM)
        )
        psAO = ctx.enter_context(
            tc.tile_pool(name="psAO", bufs=1, space=bass.MemorySpace.PSUM)
        )

        # ---- resident constants ----
        wq_sb = const.tile([128, 4, DIM], bf16)
        wk_sb = const.tile([128, 4, DIM], bf16)
        wv_sb = const.tile([128, 4, DIM], bf16)
        wout_sb = const.tile([128, 4, DIM], bf16)
        for k4 in range(4):
            nc.sync.dma_start(wq_sb[:, k4, :], wq_d[k4 * 128 : (k4 + 1) * 128, :])
            nc.sync.dma_start(wk_sb[:, k4, :], wk_d[k4 * 128 : (k4 + 1) * 128, :])
            nc.sync.dma_start(wv_sb[:, k4, :], wv_d[k4 * 128 : (k4 + 1) * 128, :])
            nc.sync.dma_start(wout_sb[:, k4, :], wout_d[k4 * 128 : (k4 + 1) * 128, :])
        e2t_sb = const.tile([128, 512], bf16)
        nc.sync.dma_start(e2t_sb[:], e2t_d[:])
        ident_sb = const.tile([128, 128], bf16)
        nc.sync.dma_start(ident_sb[:], ident_d[:])
        bout_sb = const.tile([128, 4], f32)
        nc.sync.dma_start(bout_sb[:], bout_d[:])

        def proj_gen(s, xt_t, qt, kt, v_sb):
            """Projections of superblock s; yields per PSUM group."""
            for m in range(4):
                psq = psA.tile([128, 512], f32, tag="psa", name="psq")
                for k4 in range(4):
                    nc.tensor.matmul(
                        psq[:],
                        wq_sb[:, k4, m * 128 : (m + 1) * 128],
                        xt_t[:, k4, :],
                        start=(k4 == 0),
                        stop=(k4 == 3),
                    )
                nc.scalar.copy(qt[:, m, :], psq[:])
                yield
                psk = psA.tile([128, 512], f32, tag="psa", name="psk")
                for k4 in range(4):
                    nc.tensor.matmul(
                        psk[:],
                        wk_sb[:, k4, m * 128 : (m + 1) * 128],
                        xt_t[:, k4, :],
                        start=(k4 == 0),
                        stop=(k4 == 3),
                    )
                nc.vector.tensor_copy(kt[:, m, :], psk[:])
                yield
            nc.vector.memset(v_sb[:, :, :, 64:65], 1.0)
            for mt in range(4):
                psv = psA.tile([128, 512], f32, tag="psa", name="psv")
                for k4 in range(4):
                    nc.tensor.matmul(
                        psv[:],
                        xt_t[:, k4, mt * 128 : (mt + 1) * 128],
                        wv_sb[:, k4, :],
                        start=(k4 == 0),
                        stop=(k4 == 3),
                    )
                nc.vector.tensor_copy(
                    v_sb[:, mt, :, 0:64],
                    psv[:].rearrange("p (h e) -> p h e", h=HEADS),
                )
                yield

        def p1_gen(s, qt, post_blocks):
            """Relative-position pipeline of superblock s (both blocks).
            Yields after each matmul+copy pair."""
            for b in range(2):
                blk = 2 * s + b
                p2stage = p2stpool.tile([128, HEADS, 2, 384], bf16)
                post_all = post_blocks[b]
                for ct in range(2):
                    for h in range(HEADS):
                        bp = (h % 2) * 64
                        m4 = h // 2
                        p2ps = psP.tile([128, 384], f32, tag="psp")
                        nc.tensor.matmul(
                            p2ps[:],
                            qt[
                                bp : bp + 64,
                                m4,
                                b * 256 + ct * 128 : b * 256 + ct * 128 + 128,
                            ],
                            e2t_sb[bp : bp + 64, (1 - ct) * 128 : (1 - ct) * 128 + 384],
                            tile_position=(bp, 0),
                        )
                        if (h + ct) % 2 == 0:
                            nc.scalar.copy(p2stage[:, h, ct, :], p2ps[:])
                        else:
                            nc.vector.tensor_copy(p2stage[:, h, ct, :], p2ps[:])
                        yield
                    # store + sheared read per ct half (scratch layout matches
                    # staging -> contiguous store runs)
                    nc.gpsimd.dma_start(
                        AP(
                            p2s_d,
                            blk * S_BLK + ct * 384,
                            [[S_P, 128], [2 * 384, HEADS], [1, 384]],
                        ),
                        p2stage[:, :, ct, :],
                    )
                    nc.sync.dma_start(
                        post_all[:, :, ct, :],
                        AP(
                            p2s_d,
                            blk * S_BLK + ct * 384 + 127,
                            [[S_P - 1, 128], [2 * 384, HEADS], [1, 256]],
                        ),
                    )
                    yield

        def attn_gen(st):
            """Attention phase for a staged superblock; yields per work item."""
            s, qt, kt, v_sb, post_blocks = st
            aot_sb = aotpool.tile([128, 4, 512], bf16)
            for b in range(2):
                post_all = post_blocks[b]
                ao_sb = aopool.tile([128, 2, HEADS, 64], bf16)
                for g in range(2):  # head groups of 4
                    att_group = []
                    for hp in range(2):  # head pairs (row groups alternate)
                        hpair = (4 * g + 2 * hp, 4 * g + 2 * hp + 1)
                        dts = {}
                        for h in hpair:
                            dts[h] = psD.tile(
                                [128, 2, 256], f32, tag="psd", name=f"dt{h % 2}"
                            )
                        for rt in range(2):
                            for h in hpair:
                                bp = (h % 2) * 64
                                m4 = h // 2
                                nc.tensor.matmul(
                                    dts[h][:, rt, :],
                                    kt[
                                        bp : bp + 64,
                                        m4,
                                        b * 256 + rt * 128 : b * 256 + rt * 128 + 128,
                                    ],
                                    qt[bp : bp + 64, m4, b * 256 : (b + 1) * 256],
                                    start=(rt == 0),
                                    stop=False,
                                    tile_position=(bp, 0),
                                    skip_group_check=True,
                                )
                        yield
                        for h in hpair:
                            dt_ps = dts[h]
                            for ct in range(2):
                                for rt in range(2):
                                    nc.tensor.matmul(
                                        dt_ps[:, rt, ct * 128 : ct * 128 + 128],
                                        post_all[:, h, ct, rt * 128 : rt * 128 + 128],
                                        ident_sb[:],
                                        start=False,
                                        stop=(ct == 1 and rt == 1),
                                        skip_group_check=True,
                                    )
                            att_sb = atpool.tile([128, 2, 256], bf16)
                            nc.scalar.activation(att_sb[:], dt_ps[:], EXP)
                            att_group.append(att_sb)
                            yield
                    for ct in range(2):
                        ao = psAO.tile([128, 4, 65], f32, tag="psao")
                        for hh in range(4):
                            h = 4 * g + hh
                            for rt in range(2):
                                nc.tensor.matmul(
                                    ao[:, hh, :],
                                    att_group[hh][:, rt, ct * 128 : ct * 128 + 128],
                                    v_sb[:, b * 2 + rt, h, :],
                                    start=(rt == 0),
                                    stop=(rt == 1),
                                )
                        rec = recpool.tile([128, 4], f32)
                        nc.vector.reciprocal(rec[:], ao[:, :, 64])
                        rec_b = rec[:].unsqueeze(2).to_broadcast([128, 4, 64])
                        nc.vector.tensor_mul(
                            ao_sb[:, ct, 4 * g : 4 * g + 4, :],
                            ao[:, :, 0:64],
                            rec_b,
                        )
                        yield
                # transpose AO back to feature-major
                for ct in range(2):
                    for it in range(4):
                        tp = psD.tile([128, 128], f32, tag="psd", name="tp")
                        nc.tensor.matmul(
                            tp[:],
                            ao_sb[:, ct, 2 * it : 2 * it + 2, :].rearrange(
                                "p a b -> p (a b)"
                            ),
                            ident_sb[:],
                        )
                        nc.vector.tensor_copy(
                            aot_sb[
                                :, it, b * 256 + ct * 128 : b * 256 + ct * 128 + 128
                            ],
                            tp[:],
                        )
                        if it % 2 == 1:
                            yield
            # output projection
            yt_t = ypool.tile([128, 4, 512], f32)
            for m in range(4):
                psy = psA.tile([128, 512], f32, tag="psa", name="psy")
                for k4 in range(4):
                    nc.tensor.matmul(
                        psy[:],
                        wout_sb[:, k4, m * 128 : (m + 1) * 128],
                        aot_sb[:, k4, :],
                        start=(k4 == 0),
                        stop=(k4 == 3),
                    )
                nc.scalar.activation(
                    yt_t[:, m, :], psy[:], IDENT, bias=bout_sb[:, m : m + 1]
                )
                yield
            nc.gpsimd.dma_start(
                AP(yt_d, s * 512, [[ntok, 128], [128 * ntok, 4], [1, 512]]),
                yt_t[:],
            )

        def drive(gens):
            """Round-robin the generators until all are exhausted."""
            gens = [g for g in gens if g is not None]
            while gens:
                nxt = []
                for g in gens:
                    try:
                        next(g)
                        nxt.append(g)
                    except StopIteration:
                        pass
                gens = nxt

        staged = None
        for s in range(nsb):
            xt_t = xpool.tile([128, 4, 512], bf16)
            nc.gpsimd.dma_start(
                xt_t[:],
                AP(xt_d, s * 512, [[ntok, 128], [128 * ntok, 4], [1, 512]]),
            )
            qt = qpool.tile([128, 4, 512], bf16)
            kt = kpool.tile([128, 4, 512], bf16)
            v_sb = vpool.tile([128, 4, HEADS, 65], bf16)
            post_blocks = [
                pospool.tile([128, HEADS, 2, 256], bf16, name=f"post{b}")
                for b in range(2)
            ]
            g_proj = proj_gen(s, xt_t, qt, kt, v_sb)
            g_p1 = p1_gen(s, qt, post_blocks)
            g_attn = attn_gen(staged) if staged is not None else None
            drive([g_proj, g_p1, g_attn])
            staged = (s, qt, kt, v_sb, post_blocks)
        drive([attn_gen(staged)])

    nc.compile()
    return nc


def prep_host_inputs(x, Wq, Wkv, Wout, bout, rel_emb, nb):
    """Build per-core input maps (host-side layout prep)."""
    bf = _bf16()
    scale = DH ** -0.5
    ntok = nb * C
    wq = np.ascontiguousarray((Wq * scale)).astype(bf)
    wk = np.ascontiguousarray(Wkv[:, :DIM]).astype(bf)
    wv = np.ascontiguousarray(Wkv[:, DIM:]).astype(bf)
    wout = np.ascontiguousarray(Wout).astype(bf)
    # e2t[d, j] = rel_emb[767 - j, d], j in [0, 511); duplicated on rows 64-127
    e2t = np.zeros((128, 512), dtype=bf)
    block = rel_emb[767:256:-1, :].T.astype(bf)  # [64, 511]
    e2t[0:64, 0:511] = block
    e2t[64:128, 0:511] = block
    ident = np.eye(128, dtype=np.float32).astype(bf)
    boutt = np.ascontiguousarray(bout.reshape(4, 128).T).astype(np.float32)
    in_maps = []
    for i in range(BS):
        xt = np.ascontiguousarray(x[i, :ntok, :].T).astype(bf)
        in_maps.append(
            {
                "xt": xt,
                "wq": wq,
                "wk": wk,
                "wv": wv,
                "wout": wout,
                "e2t": e2t,
                "ident": ident,
                "boutt": boutt,
            }
        )
    return in_maps


_NC_CACHE = {}


def _get_nc(nb):
    if nb not in _NC_CACHE:
        _NC_CACHE[nb] = build_nc(nb)
    return _NC_CACHE[nb]


def kernel(x, Wq, Wkv, Wout, bout, rel_emb, context_size):
    from concourse.bass_utils import run_bass_kernel_spmd

    x = np.asarray(x, dtype=np.float32)
    Wq = np.asarray(Wq, dtype=np.float32)
    Wkv = np.asarray(Wkv, dtype=np.float32)
    Wout = np.asarray(Wout, dtype=np.float32)
    bout = np.asarray(bout, dtype=np.float32)
    rel_emb = np.asarray(rel_emb, dtype=np.float32)
    assert int(context_size) == C
    assert x.shape == (BS, N_TOK, DIM)

    nb = NB_FULL
    nc = _get_nc(nb)
    in_maps = prep_host_inputs(x, Wq, Wkv, Wout, bout, rel_emb, nb)
    res = run_bass_kernel_spmd(nc, in_maps, core_ids=list(range(BS)))
    out = np.empty((BS, N_TOK, DIM), dtype=np.float32)
    for i in range(BS):
        out[i] = res.results[i]["yt"].T
    return out


# revision 31
# speedup vs baseline: 1.1157x; 1.0767x over previous
"""Trainium2 Bass kernel for block-local (sparse) attention with relative
position embeddings.

Problem (hardcoded): bs=8, n=8192, dim=512, heads=8, dim_head=64,
context_size=256 -> 32 independent 256-token blocks per batch element.

Sharding: pure data-parallel over batch -- core i computes batch element i.
Weights are replicated; no collectives.

Device pipeline (per core), v10:
  - x arrives host-pre-transposed/bf16 as xT [512, 8192]; Wq pre-scaled.
  - qT/kT feature-major (lhsT=W), v token-major (lhsT=xT), all bf16 via PE.
  - Relative position: P2 windows = q @ E2T-window per (head, c-tile); staged
    to DRAM and read back with a plain sheared 4D-AP DMA as pos[c, r].
  - dotsT[r, c] = kT.T@qT accumulates pos via transpose-as-matmul
    (identity rhs, start=False) directly in PSUM; ACT exps PSUM->SBUF giving
    attnT bf16 ready as av lhsT.
  - av rhs = [v_h | ones]: softmax denominator S[c] rides column 64;
    DVE reciprocal + broadcast multiply normalizes token-major AO;
    PE transposes AO back feature-major; out-proj adds bout via ACT bias.
  - yT [512, 8192] f32 stored; host transposes back.
  Scheduling: engines run their streams in static order, so the emitter
  interleaves three phases (projections of superblock s, pos-pipeline of s,
  attention of s-1) at work-item granularity to avoid head-of-line stalls.
"""

from contextlib import ExitStack

import numpy as np

HEADS = 8
DH = 64
DIM = 512
C = 256
MAX_POS_EMB = 512
BS = 8
N_TOK = 8192
NB_FULL = N_TOK // C  # 32 blocks

_BF16 = None


def _bf16():
    global _BF16
    if _BF16 is None:
        import ml_dtypes

        _BF16 = np.dtype(ml_dtypes.bfloat16)
    return _BF16


def build_nc(nb):
    """Build the Bass graph for nb blocks (nb*256 tokens) per core."""
    import concourse.bass as bass
    import concourse.tile as tile
    from concourse import bacc, mybir
    from concourse.ap import AP

    assert nb % 2 == 0
    nsb = nb // 2  # superblocks of 512 tokens
    ntok = nb * C

    bf16 = mybir.dt.bfloat16
    f32 = mybir.dt.float32
    EXP = mybir.ActivationFunctionType.Exp
    IDENT = mybir.ActivationFunctionType.Identity

    nc = bacc.Bacc("TRN2", target_bir_lowering=False, debug=False, num_devices=8)

    xt_d = nc.dram_tensor("xt", [DIM, ntok], bf16, kind="ExternalInput")
    wq_d = nc.dram_tensor("wq", [DIM, DIM], bf16, kind="ExternalInput")
    wk_d = nc.dram_tensor("wk", [DIM, DIM], bf16, kind="ExternalInput")
    wv_d = nc.dram_tensor("wv", [DIM, DIM], bf16, kind="ExternalInput")
    wout_d = nc.dram_tensor("wout", [DIM, DIM], bf16, kind="ExternalInput")
    e2t_d = nc.dram_tensor("e2t", [128, 512], bf16, kind="ExternalInput")
    ident_d = nc.dram_tensor("ident", [128, 128], bf16, kind="ExternalInput")
    bout_d = nc.dram_tensor("boutt", [128, 4], f32, kind="ExternalInput")
    yt_d = nc.dram_tensor("yt", [DIM, ntok], f32, kind="ExternalOutput")
    # scratch for the relative-position shear; layout [blk][p][h][ct][jl]
    # matches the SBUF staging tile so stores are fully contiguous
    p2s_d = nc.dram_tensor("p2s", [nb, 128, HEADS, 2, 384], bf16)
    S_P = HEADS * 2 * 384
    S_BLK = 128 * S_P

    with tile.TileContext(nc) as tc, ExitStack() as ctx:
        const = ctx.enter_context(tc.tile_pool(name="const", bufs=1))
        xpool = ctx.enter_context(tc.tile_pool(name="xp", bufs=2))
        qpool = ctx.enter_context(tc.tile_pool(name="qp", bufs=2))
        kpool = ctx.enter_context(tc.tile_pool(name="kp", bufs=2))
        vpool = ctx.enter_context(tc.tile_pool(name="vp", bufs=2))
        p2stpool = ctx.enter_context(tc.tile_pool(name="p2st", bufs=2))
        pospool = ctx.enter_context(tc.tile_pool(name="pos", bufs=5))
        atpool = ctx.enter_context(tc.tile_pool(name="at", bufs=8))
        recpool = ctx.enter_context(tc.tile_pool(name="rec", bufs=2))
        aopool = ctx.enter_context(tc.tile_pool(name="ao", bufs=2))
        aotpool = ctx.enter_context(tc.tile_pool(name="aot", bufs=2))
        ypool = ctx.enter_context(tc.tile_pool(name="yp", bufs=2))
        psA = ctx.enter_context(
            tc.tile_pool(name="psA", bufs=2, space=bass.MemorySpace.PSUM)
        )
        psD = ctx.enter_context(
            tc.tile_pool(name="psD", bufs=3, space=bass.MemorySpace.PSUM)
        )
        psP = ctx.enter_context(
            tc.tile_pool(name="psP", bufs=2, space=bass.MemorySpace.PSUM)
        )
        psAO = ctx.enter_context(
            tc.tile_pool(name="psAO", bufs=1, space=bass.MemorySpace.PSUM)
        )

        # ---- resident constants ----
        wq_sb = const.tile([128, 4, DIM], bf16)
        wk_sb = const.tile([128, 4, DIM], bf16)
        wv_sb = const.tile([128, 4, DIM], bf16)
        wout_sb = const.tile([128, 4, DIM], bf16)
        for k4 in range(4):
            nc.sync.dma_start(wq_sb[:, k4, :], wq_d[k4 * 128 : (k4 + 1) * 128, :])
            nc.sync.dma_start(wk_sb[:, k4, :], wk_d[k4 * 128 : (k4 + 1) * 128, :])
            nc.sync.dma_start(wv_sb[:, k4, :], wv_d[k4 * 128 : (k4 + 1) * 128, :])
            nc.sync.dma_start(wout_sb[:, k4, :], wout_d[k4 * 128 : (k4 + 1) * 128, :])
        e2t_sb = const.tile([128, 512], bf16)
        nc.sync.dma_start(e2t_sb[:], e2t_d[:])
        ident_sb = const.tile([128, 128], bf16)
        nc.sync.dma_start(ident_sb[:], ident_d[:])
        bout_sb = const.tile([128, 4], f32)
        nc.sync.dma_start(bout_sb[:], bout_d[:])

        def proj_gen(s, xt_t, qt, kt, v_sb):
            """Projections of superblock s; yields per PSUM group."""
            for m in range(4):
                psq = psA.tile([128, 512], f32, tag="psa", name="psq")
                for k4 in range(4):
                    nc.tensor.matmul(
                        psq[:],
                        wq_sb[:, k4, m * 128 : (m + 1) * 128],
                        xt_t[:, k4, :],
                        start=(k4 == 0),
                        stop=(k4 == 3),
                    )
                nc.scalar.copy(qt[:, m, :], psq[:])
                yield
                psk = psA.tile([128, 512], f32, tag="psa", name="psk")
                for k4 in range(4):
                    nc.tensor.matmul(
                        psk[:],
                        wk_sb[:, k4, m * 128 : (m + 1) * 128],
                        xt_t[:, k4, :],
                        start=(k4 == 0),
                        stop=(k4 == 3),
                    )
                nc.vector.tensor_copy(kt[:, m, :], psk[:])
                yield
            nc.vector.memset(v_sb[:, :, :, 64:65], 1.0)
            for mt in range(4):
                psv = psA.tile([128, 512], f32, tag="psa", name="psv")
                for k4 in range(4):
                    nc.tensor.matmul(
                        psv[:],
                        xt_t[:, k4, mt * 128 : (mt + 1) * 128],
                        wv_sb[:, k4, :],
                        start=(k4 == 0),
                        stop=(k4 == 3),
                    )
                nc.vector.tensor_copy(
                    v_sb[:, mt, :, 0:64],
                    psv[:].rearrange("p (h e) -> p h e", h=HEADS),
                )
                yield

        def p1_gen(s, qt, post_blocks):
            """Relative-position pipeline of superblock s (both blocks).
            Yields after each matmul+copy pair."""
            for b in range(2):
                blk = 2 * s + b
                p2stage = p2stpool.tile([128, HEADS, 2, 384], bf16)
                post_all = post_blocks[b]
                for ct in range(2):
                    for h in range(HEADS):
                        bp = (h % 2) * 64
                        m4 = h // 2
                        p2ps = psP.tile([128, 384], f32, tag="psp")
                        nc.tensor.matmul(
                            p2ps[:],
                            qt[
                                bp : bp + 64,
                                m4,
                                b * 256 + ct * 128 : b * 256 + ct * 128 + 128,
                            ],
                            e2t_sb[bp : bp + 64, (1 - ct) * 128 : (1 - ct) * 128 + 384],
                            tile_position=(bp, 0),
                        )
                        if (h + ct) % 2 == 0:
                            nc.scalar.copy(p2stage[:, h, ct, :], p2ps[:])
                        else:
                            nc.vector.tensor_copy(p2stage[:, h, ct, :], p2ps[:])
                        yield
                    # store + sheared read per ct half (scratch layout matches
                    # staging -> contiguous store runs)
                    nc.gpsimd.dma_start(
                        AP(
                            p2s_d,
                            blk * S_BLK + ct * 384,
                            [[S_P, 128], [2 * 384, HEADS], [1, 384]],
                        ),
                        p2stage[:, :, ct, :],
                    )
                    nc.sync.dma_start(
                        post_all[:, :, ct, :],
                        AP(
                            p2s_d,
                            blk * S_BLK + ct * 384 + 127,
                            [[S_P - 1, 128], [2 * 384, HEADS], [1, 256]],
                        ),
                    )
                    yield

        def attn_gen(st):
            """Attention phase for a staged superblock; yields per work item."""
            s, qt, kt, v_sb, post_blocks = st
            aot_sb = aotpool.tile([128, 4, 512], bf16)
            for b in range(2):
                post_all = post_blocks[b]
                ao_sb = aopool.tile([128, 2, HEADS, 64], bf16)
                for g in range(2):  # head groups of 4
                    att_group = []
                    for hp in range(2):  # head pairs (row groups alternate)
                        hpair = (4 * g + 2 * hp, 4 * g + 2 * hp + 1)
                        dts = {}
                        for h in hpair:
                            dts[h] = psD.tile(
                                [128, 2, 256], f32, tag="psd", name=f"dt{h % 2}"
                            )
                        for rt in range(2):
                            for h in hpair:
                                bp = (h % 2) * 64
                                m4 = h // 2
                                nc.tensor.matmul(
                                    dts[h][:, rt, :],
                                    kt[
                                        bp : bp + 64,
                                        m4,
                                        b * 256 + rt * 128 : b * 256 + rt * 128 + 128,
                                    ],
                                    qt[bp : bp + 64, m4, b * 256 : (b + 1) * 256],
                                    start=(rt == 0),
                                    stop=False,
                                    tile_position=(bp, 0),
                                    skip_group_check=True,
                                )
                        yield
                        for h in hpair:
                            dt_ps = dts[h]
                            for ct in range(2):
                                for rt in range(2):
                                    nc.tensor.matmul(
                                        dt_ps[:, rt, ct * 128 : ct * 128 + 128],
                                        post_all[:, h, ct, rt * 128 : rt * 128 + 128],
                                        ident_sb[:],
                                        start=False,
                                        stop=(ct == 1 and rt == 1),
                                        skip_group_check=True,
                                    )
                            att_sb = atpool.tile([128, 2, 256], bf16)
                            nc.scalar.activation(att_sb[:], dt_ps[:], EXP)
                            att_group.append(att_sb)
                            yield
                    for ct in range(2):
                        ao = psAO.tile([128, 4, 65], f32, tag="psao")
                        for hh in range(4):
                            h = 4 * g + hh
                            for rt in range(2):
                                nc.tensor.matmul(
                                    ao[:, hh, :],
                                    att_group[hh][:, rt, ct * 128 : ct * 128 + 128],
                                    v_sb[:, b * 2 + rt, h, :],
                                    start=(rt == 0),
                                    stop=(rt == 1),
                                )
                        rec = recpool.tile([128, 4], f32)
                        nc.vector.reciprocal(rec[:], ao[:, :, 64])
                        rec_b = rec[:].unsqueeze(2).to_broadcast([128, 4, 64])
                        nc.vector.tensor_mul(
                            ao_sb[:, ct, 4 * g : 4 * g + 4, :],
                            ao[:, :, 0:64],
                            rec_b,
                        )
                        yield
                # transpose AO back to feature-major
                for ct in range(2):
                    for it in range(4):
                        tp = psD.tile([128, 128], f32, tag="psd", name="tp")
                        nc.tensor.matmul(
                            tp[:],
                            ao_sb[:, ct, 2 * it : 2 * it + 2, :].rearrange(
                                "p a b -> p (a b)"
                            ),
                            ident_sb[:],
                        )
                        nc.vector.tensor_copy(
                            aot_sb[
                                :, it, b * 256 + ct * 128 : b * 256 + ct * 128 + 128
                            ],
                            tp[:],
                        )
                        if it % 2 == 1:
                            yield
            # output projection
            yt_t = ypool.tile([128, 4, 512], f32)
            for m in range(4):
                psy = psA.tile([128, 512], f32, tag="psa", name="psy")
                for k4 in range(4):
                    nc.tensor.matmul(
                        psy[:],
                        wout_sb[:, k4, m * 128 : (m + 1) * 128],
                        aot_sb[:, k4, :],
                        start=(k4 == 0),
                        stop=(k4 == 3),
                    )
                nc.scalar.activation(
                    yt_t[:, m, :], psy[:], IDENT, bias=bout_sb[:, m : m + 1]
                )
                yield
            nc.gpsimd.dma_start(
                AP(yt_d, s * 512, [[ntok, 128], [128 * ntok, 4], [1, 512]]),
                yt_t[:],
            )

        def drive(gens, weights=None):
            """Weighted round-robin of the generators until exhausted."""
            pairs = [
                (g, (weights or {}).get(i, 1))
                for i, g in enumerate(gens)
                if g is not None
            ]
            while pairs:
                nxt = []
                for g, w in pairs:
                    alive = True
                    for _ in range(w):
                        try:
                            next(g)
                        except StopIteration:
                            alive = False
                            break
                    if alive:
                        nxt.append((g, w))
                pairs = nxt

        staged = None
        for s in range(nsb):
            xt_t = xpool.tile([128, 4, 512], bf16)
            nc.gpsimd.dma_start(
                xt_t[:],
                AP(xt_d, s * 512, [[ntok, 128], [128 * ntok, 4], [1, 512]]),
            )
            qt = qpool.tile([128, 4, 512], bf16)
            kt = kpool.tile([128, 4, 512], bf16)
            v_sb = vpool.tile([128, 4, HEADS, 65], bf16)
            post_blocks = [
                pospool.tile([128, HEADS, 2, 256], bf16, name=f"post{b}")
                for b in range(2)
            ]
            g_proj = proj_gen(s, xt_t, qt, kt, v_sb)
            g_p1 = p1_gen(s, qt, post_blocks)
            g_attn = attn_gen(staged) if staged is not None else None
            drive([g_proj, g_p1, g_attn], weights={2: 2})
            staged = (s, qt, kt, v_sb, post_blocks)
        drive([attn_gen(staged)])

    nc.compile()
    return nc


def prep_host_inputs(x, Wq, Wkv, Wout, bout, rel_emb, nb):
    """Build per-core input maps (host-side layout prep)."""
    bf = _bf16()
    scale = DH ** -0.5
    ntok = nb * C
    wq = np.ascontiguousarray((Wq * scale)).astype(bf)
    wk = np.ascontiguousarray(Wkv[:, :DIM]).astype(bf)
    wv = np.ascontiguousarray(Wkv[:, DIM:]).astype(bf)
    wout = np.ascontiguousarray(Wout).astype(bf)
    # e2t[d, j] = rel_emb[767 - j, d], j in [0, 511); duplicated on rows 64-127
    e2t = np.zeros((128, 512), dtype=bf)
    block = rel_emb[767:256:-1, :].T.astype(bf)  # [64, 511]
    e2t[0:64, 0:511] = block
    e2t[64:128, 0:511] = block
    ident = np.eye(128, dtype=np.float32).astype(bf)
    boutt = np.ascontiguousarray(bout.reshape(4, 128).T).astype(np.float32)
    in_maps = []
    for i in range(BS):
        xt = np.ascontiguousarray(x[i, :ntok, :].T).astype(bf)
        in_maps.append(
            {
                "xt": xt,
                "wq": wq,
                "wk": wk,
                "wv": wv,
                "wout": wout,
                "e2t": e2t,
                "ident": ident,
                "boutt": boutt,
            }
        )
    return in_maps


_NC_CACHE = {}


def _get_nc(nb):
    if nb not in _NC_CACHE:
        _NC_CACHE[nb] = build_nc(nb)
    return _NC_CACHE[nb]


def kernel(x, Wq, Wkv, Wout, bout, rel_emb, context_size):
    from concourse.bass_utils import run_bass_kernel_spmd

    x = np.asarray(x, dtype=np.float32)
    Wq = np.asarray(Wq, dtype=np.float32)
    Wkv = np.asarray(Wkv, dtype=np.float32)
    Wout = np.asarray(Wout, dtype=np.float32)
    bout = np.asarray(bout, dtype=np.float32)
    rel_emb = np.asarray(rel_emb, dtype=np.float32)
    assert int(context_size) == C
    assert x.shape == (BS, N_TOK, DIM)

    nb = NB_FULL
    nc = _get_nc(nb)
    in_maps = prep_host_inputs(x, Wq, Wkv, Wout, bout, rel_emb, nb)
    res = run_bass_kernel_spmd(nc, in_maps, core_ids=list(range(BS)))
    out = np.empty((BS, N_TOK, DIM), dtype=np.float32)
    for i in range(BS):
        out[i] = res.results[i]["yt"].T
    return out
